# revision 1
# baseline (speedup 1.0000x reference)
"""Instruction-minimal DigitCaps kernel for 8 TRN2 cores.

Measurement on this system showed wall time scales with instruction count
(~250ns/instr) rather than bytes or FLOPs, so every loop is batched into
the widest legal instruction:

  Phase A : dma_group=8 route DMA (36 DMAs), 2-route PSUM batches (72
            PSUM->SBUF copies), s0 via 16 tensor_reduces (no per-route adds).
  s-pass  : per-o tensor_tensor mult + reduce-X (32 instr vs 160).
  agree   : per-o tensor_tensor mult-add chain + 3 ones-matmul partition
            reduces (37 instr vs 170 tiny PE matmuls).
  softmax : local-max-shifted exp with a small overlapped AllReduce(max)
            folded in by an exp(m_loc-M) payload rescale (input-agnostic;
            jax.random bits differ across backends, so no static bounds).
  u_hat   : fp32 end to end.  The routing amplifies ~5e-4 perturbations of
            the agreement logits into O(30%) output changes on input draws
            with near-tied routes (observed across jax backends), so no
            reduced-precision storage anywhere.

Five collectives total: one fp32 [B+1,160] AllReduce(add) per routing
iteration carrying the s numerator + softmax denominator row, plus two
small overlapped AllReduce(max) ops for softmax stability.
"""

import numpy as np

B, R, C, O, I = 128, 1152, 10, 16, 338
N_CORES = 8
R_LOC = R // N_CORES          # 144
IP = 384                      # i padded to 3*128
NCH = IP // 128               # 3 contraction chunks
CO = C * O                    # 160
G = 8                         # routes per DMA
GP = 2                        # routes per PSUM tile

_CACHE = {}


def _build_nc(r_loc=R_LOC, n_cores=N_CORES, reps=1, stages=99, no_cc=False,
              agree_mode="t4", nch=NCH, skip_copy=False, interleave=0):
    import concourse.tile as tile
    from concourse import bacc, mybir

    f32 = mybir.dt.float32
    f16 = mybir.dt.float16
    nc = bacc.Bacc("TRN2", target_bir_lowering=False, debug=False,
                   enable_asserts=False, num_devices=n_cores)

    xk = nc.dram_tensor("xk", [r_loc // G, 128, G, NCH, B], f32,
                        kind="ExternalInput")
    wk = nc.dram_tensor("wk", [r_loc // G, 128, G, NCH, CO], f32,
                        kind="ExternalInput")
    out = nc.dram_tensor("out", [B, CO], f32, kind="ExternalOutput")

    groups = [list(range(n_cores))]

    with tile.TileContext(nc) as tc:
        with (
            tc.tile_pool(name="u", bufs=1) as u_pool,
            tc.tile_pool(name="stream", bufs=2) as stream,
            tc.tile_pool(name="small", bufs=1) as small,
            tc.tile_pool(name="t3p", bufs=2) as t3_pool,
            tc.tile_pool(name="upsum",
                         bufs=(2 if interleave == 3 else
                               1 if interleave == 2 else 4),
                         space="PSUM") as upsum_pool,
            tc.tile_pool(name="apsum", bufs=2, space="PSUM") as apsum_pool,
            tc.tile_pool(name="wbc", bufs=2, space="PSUM") as wbc_pool,
            tc.tile_pool(name="dram", bufs=1, space="DRAM") as dram,
        ):
            u_sb = u_pool.tile([B, O, C, r_loc], f32)    # u_hat [b,o,c,r]
            s0_acc = small.tile([B, CO], f32)
            ones_sb = small.tile([1, 128], f32)
            nc.vector.memset(ones_sb[:], 1.0)
            ones_col = small.tile([B, 1], f32)
            nc.vector.memset(ones_col[:], 1.0)
            b_sb = small.tile([1, C, r_loc], f32)
            w_sb = small.tile([B, C, r_loc], f32)
            s_sb = small.tile([B, CO], f32)
            v_sb = small.tile([B, CO], f32)
            d_all = small.tile([B, C], f32)
            rd_sb = small.tile([B, C], f32)
            agr_t = small.tile([B, C, r_loc], f32)
            b_shift = small.tile([1, C, r_loc], f32)
            mx_loc = small.tile([1, 16], f32)
            mx_row = small.tile([1, C], f32)
            sc_row = small.tile([1, C], f32)
            d2 = small.tile([1, C], f32)
            sq_t = small.tile([B, CO], f32)
            rden_t = small.tile([B, CO], f32)
            sabs_t = small.tile([B, CO], f32)
            out_sb = small.tile([B, CO], f32)

            for rep in range(reps):
                nc.vector.memset(b_sb[:], 0.0)
                # ---------- Phase A ----------
                for rg in range(r_loc // G):
                    x_t = stream.tile([128, G, NCH, B], f32, tag="x")
                    w_t = stream.tile([128, G, NCH, CO], f32, tag="w")
                    nc.sync.dma_start(x_t[:], xk[rg])
                    nc.sync.dma_start(w_t[:], wk[rg])
                    if interleave == 3:
                        # no middle (start=F,stop=F) matmuls: ch0+ch1 as a
                        # 2-chain into tile A, ch2 standalone into tile B,
                        # evacuate with one DVE add A+B -> u_sb (fp16)
                        for rp in range(G // GP):
                            upa = upsum_pool.tile([B, GP, CO], f32, tag="ua")
                            upb = upsum_pool.tile([B, GP, CO], f32, tag="ub")
                            for q in range(GP):
                                g = rp * GP + q
                                nc.tensor.matmul(upa[:, q, :],
                                                 x_t[:, g, 0, :],
                                                 w_t[:, g, 0, :],
                                                 start=True, stop=False)
                                nc.tensor.matmul(upa[:, q, :],
                                                 x_t[:, g, 1, :],
                                                 w_t[:, g, 1, :],
                                                 start=False, stop=True)
                                nc.tensor.matmul(upb[:, q, :],
                                                 x_t[:, g, 2, :],
                                                 w_t[:, g, 2, :],
                                                 start=True, stop=True)
                            r0 = rg * G + rp * GP
                            if not skip_copy:
                                nc.scalar.activation(
                                    u_sb[:, :, :, r0:r0 + GP],
                                    upa[:].rearrange("p q (o c) -> p o c q",
                                                     o=O),
                                    mybir.ActivationFunctionType.Copy)
                                nc.vector.tensor_tensor(
                                    u_sb[:, :, :, r0:r0 + GP],
                                    u_sb[:, :, :, r0:r0 + GP],
                                    upb[:].rearrange("p q (o c) -> p o c q",
                                                     o=O),
                                    mybir.AluOpType.add)
                        continue
                    if interleave == 2:
                        # interleave accumulation chains across all 4 PSUM
                        # tiles of the group: successive matmuls into the
                        # same bank are 8 issues apart, hiding the RAW drain
                        upss = [upsum_pool.tile([B, GP, CO], f32,
                                                tag=f"up{rp}")
                                for rp in range(G // GP)]
                        for ch in range(nch):
                            for rp in range(G // GP):
                                for q in range(GP):
                                    g = rp * GP + q
                                    nc.tensor.matmul(upss[rp][:, q, :],
                                                     x_t[:, g, ch, :],
                                                     w_t[:, g, ch, :],
                                                     start=(ch == 0),
                                                     stop=(ch == nch - 1))
                        for rp in range(G // GP):
                            r0 = rg * G + rp * GP
                            if not skip_copy:
                                nc.scalar.activation(
                                    u_sb[:, :, :, r0:r0 + GP],
                                    upss[rp][:].rearrange(
                                        "p q (o c) -> p o c q", o=O),
                                    mybir.ActivationFunctionType.Copy)
                        continue
                    for rp in range(G // GP):
                        ups = upsum_pool.tile([B, GP, CO], f32)
                        if interleave == 1:
                            for ch in range(nch):
                                for q in range(GP):
                                    g = rp * GP + q
                                    nc.tensor.matmul(ups[:, q, :],
                                                     x_t[:, g, ch, :],
                                                     w_t[:, g, ch, :],
                                                     start=(ch == 0),
                                                     stop=(ch == nch - 1))
                        else:
                            for q in range(GP):
                                g = rp * GP + q
                                for ch in range(nch):
                                    nc.tensor.matmul(ups[:, q, :],
                                                     x_t[:, g, ch, :],
                                                     w_t[:, g, ch, :],
                                                     start=(ch == 0),
                                                     stop=(ch == nch - 1))
                        r0 = rg * G + rp * GP
                        if not skip_copy:
                            nc.scalar.activation(
                                u_sb[:, :, :, r0:r0 + GP],
                                ups[:].rearrange("p q (o c) -> p o c q", o=O),
                                mybir.ActivationFunctionType.Copy)
                # s0 = sum_r u_hat (16 reduces over the resident tile)
                for o in range(O):
                    nc.vector.tensor_reduce(
                        s0_acc[:, o * C:(o + 1) * C], u_sb[:, o, :, :],
                        mybir.AxisListType.X, mybir.AluOpType.add)

                # ---------- helpers ----------
                def all_reduce(sb_src, with_d, it):
                    rows = B + 1 if with_d else B
                    ar_in = dram.tile([rows, CO], f32, name=f"ari{rep}_{it}")
                    ar_out = dram.tile([rows, CO], f32, addr_space="Shared",
                                       name=f"aro{rep}_{it}")
                    nc.sync.dma_start(ar_in[0:B, :], sb_src[:])
                    if with_d:
                        nc.sync.dma_start(ar_in[B:B + 1, 0:C], d_all[0:1, :])
                    if no_cc:
                        nc.sync.dma_start(ar_out[:, :], ar_in[:, :])
                    else:
                        nc.gpsimd.collective_compute(
                            "AllReduce", mybir.AluOpType.add,
                            replica_groups=groups,
                            ins=[ar_in.opt()], outs=[ar_out.opt()])
                    nc.sync.dma_start(s_sb[:], ar_out[0:B, :])
                    if with_d:
                        nc.sync.dma_start(d2[:], ar_out[B:B + 1, 0:C])
                        dps = wbc_pool.tile([B, 512], f32, tag="wbc",
                                            name=f"dps{rep}_{it}")
                        nc.tensor.matmul(dps[:, 0:C], ones_sb[:], d2[:])
                        nc.vector.reciprocal(rd_sb[:], dps[:, 0:C])
                        # s *= 1/d  (one op, rd broadcast over o)
                        nc.vector.tensor_tensor(
                            s_sb[:].rearrange("p (o c) -> p o c", o=O),
                            s_sb[:].rearrange("p (o c) -> p o c", o=O),
                            rd_sb[:].unsqueeze(1).broadcast_to((B, O, C)),
                            mybir.AluOpType.mult)
                    else:
                        nc.vector.tensor_scalar_mul(s_sb[:], s_sb[:],
                                                    1.0 / (r_loc * n_cores))

                def squash():
                    # v = s*|s| / (1+s^2):  sq=s*s; rden=1/(1+sq);
                    # sabs=|s| (ACT); v=(s*sabs)*rden
                    nc.vector.tensor_tensor(sq_t[:], s_sb[:], s_sb[:],
                                            mybir.AluOpType.mult)
                    nc.vector.tensor_scalar_add(sq_t[:], sq_t[:], 1.0)
                    nc.vector.reciprocal(rden_t[:], sq_t[:])
                    nc.scalar.activation(sabs_t[:], s_sb[:],
                                         mybir.ActivationFunctionType.Abs)
                    nc.vector.tensor_tensor(sabs_t[:], s_sb[:], sabs_t[:],
                                            mybir.AluOpType.mult)
                    nc.vector.tensor_tensor(v_sb[:], sabs_t[:], rden_t[:],
                                            mybir.AluOpType.mult)

                def agreement(it):
                    # b += (1/B) sum_b sum_o u*v   (all fp32: the routing is
                    # chaotically sensitive to b perturbations on some input
                    # draws, so no reduced precision anywhere in this path)
                    for o in range(O):
                        vb = (v_sb[:, o * C:(o + 1) * C]
                              .unsqueeze(2).broadcast_to((B, C, r_loc)))
                        t3 = t3_pool.tile([B, C, r_loc], f32, tag="t3",
                                          name=f"at3_{rep}_{it}_{o}")
                        nc.vector.tensor_tensor(t3[:], u_sb[:, o, :, :],
                                                vb, mybir.AluOpType.mult)
                        if o == 0:
                            nc.vector.tensor_copy(agr_t[:], t3[:])
                        else:
                            nc.vector.tensor_tensor(agr_t[:], agr_t[:],
                                                    t3[:],
                                                    mybir.AluOpType.add)
                    a_flat = agr_t[:].rearrange("p c r -> p (c r)")
                    b_flat = b_sb[:].rearrange("p c r -> p (c r)")
                    off = 0
                    while off < C * r_loc:
                        n = min(512, C * r_loc - off)
                        aps = apsum_pool.tile([1, 512], f32, tag="a",
                                              name=f"a{rep}_{it}_{off}")
                        nc.tensor.matmul(aps[:, 0:n], ones_col[:],
                                         a_flat[:, off:off + n])
                        nc.vector.scalar_tensor_tensor(
                            b_flat[0:1, off:off + n], aps[:, 0:n], 1.0 / B,
                            b_flat[0:1, off:off + n],
                            mybir.AluOpType.mult, mybir.AluOpType.add)
                        off += n

                def weights_and_s(it):
                    # softmax numerator/denominator with the LOCAL per-capsule
                    # max subtracted; the cross-core max arrives via a small
                    # AllReduce(max) that overlaps exp + the s-pass, and is
                    # folded in afterwards by rescaling the AR payload with
                    # exp(m_loc - M) (softmax-invariant, input-agnostic).
                    nc.vector.memset(mx_loc[:], -1e30)
                    nc.vector.tensor_reduce(mx_loc[0:1, 0:C], b_sb[0:1, :, :],
                                            mybir.AxisListType.X,
                                            mybir.AluOpType.max)
                    mx_in = dram.tile([1, 16], f32, name=f"mxi{rep}_{it}")
                    mx_out = dram.tile([1, 16], f32, addr_space="Shared",
                                       name=f"mxo{rep}_{it}")
                    nc.sync.dma_start(mx_in[:], mx_loc[:])
                    if no_cc:
                        nc.sync.dma_start(mx_out[:, :], mx_in[:, :])
                    else:
                        nc.gpsimd.collective_compute(
                            "AllReduce", mybir.AluOpType.max,
                            replica_groups=groups,
                            ins=[mx_in.opt()], outs=[mx_out.opt()])
                    nc.sync.dma_start(mx_row[:], mx_out[0:1, 0:C])
                    # b_shift = b - m_loc  (one broadcast subtract)
                    nc.vector.tensor_tensor(
                        b_shift[:], b_sb[:],
                        mx_loc[0:1, 0:C].unsqueeze(2)
                        .broadcast_to((1, C, r_loc)),
                        mybir.AluOpType.subtract)
                    # w = exp(b_shift) broadcast to all partitions via PE
                    bs_flat = b_shift[:].rearrange("p c r -> p (c r)")
                    w_flat = w_sb[:].rearrange("p c r -> p (c r)")
                    off = 0
                    while off < C * r_loc:
                        n = min(512, C * r_loc - off)
                        wb = wbc_pool.tile([B, 512], f32, tag="wbc",
                                           name=f"wb{rep}_{it}_{off}")
                        nc.tensor.matmul(wb[:, 0:n], ones_sb[:],
                                         bs_flat[:, off:off + n])
                        nc.scalar.activation(w_flat[:, off:off + n],
                                             wb[:, 0:n],
                                             mybir.ActivationFunctionType.Exp)
                        off += n
                    nc.vector.tensor_reduce(d_all[:], w_sb[:],
                                            mybir.AxisListType.X,
                                            mybir.AluOpType.add)
                    # s numerator per o: one mult + one reduce (w <= 1 so the
                    # fp32 scratch never overflows)
                    for o in range(O):
                        t3 = t3_pool.tile([B, C, r_loc], f32, tag="t3",
                                          name=f"st3_{rep}_{it}_{o}")
                        nc.vector.tensor_tensor(t3[:], u_sb[:, o, :, :],
                                                w_sb[:], mybir.AluOpType.mult)
                        nc.vector.tensor_reduce(
                            s_sb[:, o * C:(o + 1) * C], t3[:],
                            mybir.AxisListType.X, mybir.AluOpType.add)
                    # fold in the global max: payload *= exp(m_loc - M);
                    # waits on the max-AR, which overlapped the work above
                    nc.vector.tensor_tensor(sc_row[:], mx_loc[0:1, 0:C],
                                            mx_row[:],
                                            mybir.AluOpType.subtract)
                    nc.scalar.activation(sc_row[:], sc_row[:],
                                         mybir.ActivationFunctionType.Exp)
                    scps = wbc_pool.tile([B, 512], f32, tag="wbc",
                                         name=f"scps{rep}_{it}")
                    nc.tensor.matmul(scps[:, 0:C], ones_sb[:], sc_row[:])
                    nc.vector.tensor_tensor(
                        s_sb[:].rearrange("p (o c) -> p o c", o=O),
                        s_sb[:].rearrange("p (o c) -> p o c", o=O),
                        scps[:, 0:C].unsqueeze(1).broadcast_to((B, O, C)),
                        mybir.AluOpType.mult)
                    nc.vector.tensor_tensor(d_all[:], d_all[:],
                                            scps[:, 0:C],
                                            mybir.AluOpType.mult)

                # ---------- iterations ----------
                if stages >= 1:
                    all_reduce(s0_acc, with_d=False, it=0)
                    squash()
                else:
                    nc.vector.tensor_copy(v_sb[:], s0_acc[:])
                if stages >= 2:
                    agreement(0)
                if stages >= 3:
                    weights_and_s(1)
                if stages >= 4:
                    all_reduce(s_sb, with_d=True, it=1)
                    squash()
                if stages >= 5:
                    agreement(1)
                if stages >= 6:
                    weights_and_s(2)
                    all_reduce(s_sb, with_d=True, it=2)
                    squash()
                # ---------- output ----------
                nc.vector.tensor_copy(
                    out_sb[:].rearrange("p (c o) -> p c o", c=C),
                    v_sb[:].rearrange("p (o c) -> p c o", o=O))
                nc.sync.dma_start(out[:, :], out_sb[:])

    nc.compile()
    return nc


def _make_runner(nc):
    import jax
    from jax.sharding import Mesh, PartitionSpec, NamedSharding
    from jax.experimental.shard_map import shard_map
    from concourse import bass2jax, mybir
    from concourse.bass2jax import _bass_exec_p
    from concourse.mybir import MemoryLocationSet

    bass2jax.install_neuronx_cc_hook()
    partition_name = nc.partition_id_tensor.name if nc.partition_id_tensor else None
    in_names, out_names, out_avals, zero_outs = [], [], [], []
    for alloc in nc.m.functions[0].allocations:
        if not isinstance(alloc, MemoryLocationSet):
            continue
        name = alloc.memorylocations[0].name
        if alloc.kind == "ExternalInput":
            if name != partition_name:
                in_names.append(name)
        elif alloc.kind == "ExternalOutput":
            out_names.append(name)
            shape = tuple(alloc.tensor_shape)
            dtype = mybir.dt.np(alloc.dtype)
            out_avals.append(jax.core.ShapedArray(shape, dtype))
            zero_outs.append(np.zeros(shape, dtype))
    n_params = len(in_names)
    all_in_names = list(in_names) + out_names
    if partition_name is not None:
        all_in_names.append(partition_name)

    def _body(*args):
        operands = list(args)
        if partition_name is not None:
            operands.append(bass2jax.partition_id_tensor())
        outs = _bass_exec_p.bind(
            *operands, out_avals=tuple(out_avals), in_names=tuple(all_in_names),
            out_names=tuple(out_names), lowering_input_output_aliases=(),
            sim_require_finite=True, sim_require_nnan=True, nc=nc)
        return tuple(outs)

    devices = jax.devices()[:N_CORES]
    mesh = Mesh(np.asarray(devices), ("core",))
    in_specs = (PartitionSpec("core"),) * (n_params + len(out_names))
    out_specs = (PartitionSpec("core"),) * len(out_names)
    sharded = jax.jit(
        shard_map(_body, mesh=mesh, in_specs=in_specs, out_specs=out_specs,
                  check_rep=False),
        keep_unused=True)
    sharding = NamedSharding(mesh, PartitionSpec("core"))

    class Runner:
        _sharded = staticmethod(sharded)

        def put(self, in_maps):
            import jax as _jax
            concat = [np.concatenate([np.asarray(in_maps[c][nm])
                                      for c in range(N_CORES)], axis=0)
                      for nm in in_names]
            dz = [_jax.device_put(
                np.zeros((N_CORES * z.shape[0], *z.shape[1:]), z.dtype), sharding)
                for z in zero_outs]
            return [_jax.device_put(a, sharding) for a in concat] + dz

        def run(self, dev_args):
            import jax as _jax
            outs = sharded(*dev_args)
            _jax.block_until_ready(outs)
            return outs

        def results(self, outs):
            return [{nm: np.asarray(outs[i]).reshape(N_CORES, *out_avals[i].shape)[c]
                     for i, nm in enumerate(out_names)}
                    for c in range(N_CORES)]

    return Runner()


def _prep_shards(x, W):
    """Full inputs -> per-core in_maps: [r_loc//G, 128, G, NCH, B/CO] fp32."""
    x = np.asarray(x, dtype=np.float32)
    W = np.asarray(W, dtype=np.float32)
    in_maps = []
    for k in range(N_CORES):
        rs = slice(k * R_LOC, (k + 1) * R_LOC)
        xs = np.zeros((R_LOC, IP, B), dtype=np.float32)
        xs[:, :I, :] = np.transpose(x[:, rs, :], (1, 2, 0))
        ws = np.zeros((R_LOC, IP, CO), dtype=np.float32)
        ws[:, :I, :] = np.transpose(W[rs], (0, 3, 2, 1)).reshape(R_LOC, I, CO)
        xs = (xs.reshape(R_LOC // G, G, NCH, 128, B)
              .transpose(0, 3, 1, 2, 4))
        ws = (ws.reshape(R_LOC // G, G, NCH, 128, CO)
              .transpose(0, 3, 1, 2, 4))
        in_maps.append({
            "xk": np.ascontiguousarray(xs),
            "wk": np.ascontiguousarray(ws),
        })
    return in_maps


def _get_state():
    if "runner" not in _CACHE:
        nc = _build_nc()
        _CACHE["nc"] = nc
        _CACHE["runner"] = _make_runner(nc)
    return _CACHE["runner"]


def kernel(x, W):
    runner = _get_state()
    in_maps = _prep_shards(x, W)
    dev_args = runner.put(in_maps)
    outs = runner.run(dev_args)
    res = runner.results(outs)
    v = res[0]["out"]                       # [B, (c,o)]
    return v.reshape(B, C, O, 1)

